# revision 55
# baseline (speedup 1.0000x reference)
"""Trainium2 Bass kernel for fused sparse-attention block (nn_Attention_790273982568).

Full (unsharded) inputs in, full output out. Internally: tensor-parallel over
heads across 8 NeuronCores — each core owns 4 Q heads + 1 KV head (wqkv rows)
and 512 output columns of wo (rows of wo), with per-head on-device AllGathers
of the attention outputs (overlapped with attention) before the output
projection.

Key implementation notes:
- All [tok, hd] -> [hd, tok] transposes run on the DMA crossbar
  (dma_start_transpose), not the PE array.
- QKV projection interleaves the two token tiles of a pair inside one
  k-sweep so the DMA-bound bootstrap keeps the PE fed.
- Per-head LayerNorm stats are computed batched ([P, 5] per token tile:
  one Square, one Sqrt, one reciprocal) and the LN scale/bias plus rope
  cos/sin are folded into host-precomputed per-token tables.
- Attention spans are emitted j=0-first across heads (bridges the
  phase-1 -> phase-2 transition), then head-major so per-head AllGathers
  still overlap the remaining attention.
"""

import os
import sys

import numpy as np

for _p in ("/opt/trn_rl_repo", "/root/.axon_site/_ro/trn_rl_repo"):
    if _p not in sys.path and os.path.isdir(_p):
        sys.path.append(_p)

import ml_dtypes  # noqa: E402

import bass_rust as _bass_rust  # noqa: E402
import concourse.bass as bass  # noqa: E402
from concourse import bacc  # noqa: E402
import concourse.mybir as mybir  # noqa: E402
import concourse.tile as tile  # noqa: E402
from concourse.bass import ds, ts  # noqa: E402
from concourse.bass_utils import run_bass_kernel_spmd  # noqa: E402

# Problem shapes (hardcoded per spec)
T = 2048
DIM = 4096
HD = 128
NH = 32
NKV = 8
NCORES = 8
QH = NH // NCORES          # 4 q heads per core
FEAT = (QH + 2) * HD       # 768 qkv features per core
OUTC = DIM // NCORES       # 512 output columns per core
P = 128
NT = T // P                # 16 token tiles
KC = DIM // P              # 32 contraction chunks
QSPAN = 512
NQS = T // QSPAN           # 4 q spans
HALF = HD // 2
EPS = 1e-5
THETA = 10000.0
SCALE = 1.0 / float(np.sqrt(HD))

BF16 = mybir.dt.bfloat16
F32 = mybir.dt.float32
AX = mybir.AxisListType
ALU = mybir.AluOpType
ACTF = mybir.ActivationFunctionType

_PROGRAM_CACHE = {}


def _build_body(nc, aps):
    xT = aps["xT"]
    wqkvT = aps["wqkvT"]
    woT = aps["woT"]
    ropeT = aps["ropeT"]
    biasT = aps.get("biasT")
    masks = aps["masks"]
    ag_in = aps["ag_in"]
    ag_out = aps["ag_out"]       # [QH, NCORES*P, T]
    outT = aps["outT"]
    tc = aps["tc"]
    has_bias = aps["has_bias"]

    with (
        tc.tile_pool(name="consts", bufs=1) as consts,
        # pre-reserved pools for the early attention spans: their tiles must
        # NOT reuse SBUF whose last readers are phase-1's deferred rope ops,
        # or the exps pick up a write-after-read hazard on the rope tail
        tc.tile_pool(name="p2e", bufs=2) as p2e,
        tc.tile_pool(name="p2s", bufs=26) as p2s,
    ):
        masks_sb = consts.tile([P, 4, QSPAN], BF16, tag="masks")
        ident_sb = consts.tile([P, P], BF16, tag="ident")
        nc.gpsimd.dma_start(ident_sb[:], aps["ident"][:, :])

        # persistent activation strips
        qkT = consts.tile([P, QH + 1, T], BF16, tag="qkT")       # [hd, head, tok]
        vaug = consts.tile([P, NT, HD + 1], BF16, tag="vaug")    # [ktok%, ktile, hd+1]

        # ---------------- Phase 1: QKV projection + LN + RoPE ----------------
        with (
            tc.tile_pool(name="wq", bufs=1) as wq_pool,
            tc.tile_pool(name="p1", bufs=4) as p1,
            tc.tile_pool(name="p1sq", bufs=2) as p1sq,
            tc.tile_pool(name="px", bufs=2) as px,
            tc.tile_pool(name="p1s", bufs=6) as p1s,
            tc.tile_pool(name="prq", bufs=3) as prq_pool,
            tc.tile_pool(name="psum1", bufs=4, space="PSUM") as psum1,
        ):
            # 8-slot ring of per-tile rope tables (saves 16KB/partition)
            NSLOT = 8
            rope_sb = wq_pool.tile([P, NSLOT, 8, HALF], F32, tag="rope")
            if has_bias:
                bias_sb = wq_pool.tile([P, NSLOT, 4, HALF], F32, tag="biasT")

            def load_xt(pair, eng=None):
                tiles = px.tile([P, KC, 2 * P], BF16, tag="xt", name=f"xt_{pair}")
                for g in range(KC // 4):
                    (eng or nc.sync).dma_start(
                        tiles[:, ds(4 * g, 4), :],
                        xT[ds(4 * g * P, 4 * P), ds(pair * 2 * P, 2 * P)].rearrange(
                            "(k p) c -> p k c", p=P
                        ),
                    )
                return tiles

            def load_tables(pair, eng):
                slot = (2 * pair) % NSLOT
                eng.dma_start(
                    rope_sb[:, ds(slot, 2), :, :],
                    ropeT[:, ds(2 * pair, 2), :, :],
                )
                if has_bias:
                    eng.dma_start(
                        bias_sb[:, ds(slot, 2), :, :],
                        biasT[:, ds(2 * pair, 2), :, :],
                    )

            # stripe x/weight chunks across both HWDGE rings, k-interleaved,
            # so matmul k can start as soon as chunk k has landed
            xt0 = px.tile([P, KC, 2 * P], BF16, tag="xt", name="xt_0")
            wqkvT_sb = wq_pool.tile([P, KC, FEAT], BF16, tag="wqkvT")
            # single-chunk groups first so matmul k=0 starts ~3x sooner
            groups = [(k, 1) for k in range(4)] + [
                (g0, 4) for g0 in range(4, KC, 4)
            ]
            for gi, (g0, gn) in enumerate(groups):
                e0, e1 = (nc.sync, nc.scalar) if gi % 2 == 0 else (nc.scalar, nc.sync)
                e0.dma_start(
                    wqkvT_sb[:, ds(g0, gn), :],
                    wqkvT[ds(g0 * P, gn * P), :].rearrange("(k p) f -> p k f", p=P),
                )
                e1.dma_start(
                    xt0[:, ds(g0, gn), :],
                    xT[ds(g0 * P, gn * P), ds(0, 2 * P)].rearrange(
                        "(k p) c -> p k c", p=P
                    ),
                )
            load_tables(0, nc.gpsimd)
            nc.gpsimd.dma_start(masks_sb[:], masks[:, :, :])
            xt_cache = {0: xt0}

            def emit_stats(tt, pq):
                # v slice straight to vaug (round f32->bf16 exactly once).
                # The last pair's copies ride ACT so the transition-critical
                # LN stats aren't stuck behind the DVE queue.
                last = tt >= NT - 2
                cp = nc.scalar.copy if last else nc.vector.tensor_copy
                cp(vaug[:, tt, 0:HD], pq[:, 640:FEAT])
                nc.vector.memset(vaug[:, tt, HD : HD + 1], 1.0)
                # q/k slices as bf16 (match reference's bf16 xqkv)
                xq = p1.tile([P, 5, HD], BF16, tag="xq", name=f"xq_{tt}")
                cp(xq[:], pq[:, 0 : 5 * HD])

                # batched LN stats for all 5 heads: [P, 5]
                sq = p1sq.tile([P, 5, HD], F32, tag="sq")
                nc.scalar.activation(sq[:], xq[:], ACTF.Square)
                s1 = p1s.tile([P, 5], F32, tag="s1")
                nc.vector.reduce_sum(s1[:], xq[:], axis=AX.X)
                ssq = p1s.tile([P, 5], F32, tag="ssq")
                nc.vector.reduce_sum(ssq[:], sq[:], axis=AX.X)
                negmu = p1s.tile([P, 5], F32, tag="negmu")
                nc.vector.tensor_scalar_mul(negmu[:], s1[:], -1.0 / HD)
                mu2 = p1s.tile([P, 5], F32, tag="mu2")
                nc.vector.tensor_mul(mu2[:], negmu[:], negmu[:])
                v1 = p1s.tile([P, 5], F32, tag="v1")
                nc.vector.tensor_scalar(
                    v1[:], ssq[:], 1.0 / HD, EPS, op0=ALU.mult, op1=ALU.add
                )
                varb = p1s.tile([P, 5], F32, tag="varb")
                nc.vector.tensor_sub(varb[:], v1[:], mu2[:])
                std = p1s.tile([P, 5], F32, tag="std")
                _si = nc.scalar.activation(std[:], varb[:], ACTF.Sqrt)
                aps["last_sqrt"] = _si.ins
                rstd = p1s.tile([P, 5], F32, tag="rstd", name=f"rstd_{tt}")
                nc.vector.reciprocal(rstd[:], std[:])
                nbias = p1s.tile([P, 5], F32, tag="nbias", name=f"nb_{tt}")
                nc.vector.tensor_mul(nbias[:], negmu[:], rstd[:])
                return xq, rstd, nbias

            def emit_rope(tt, xq, rstd, nbias):
                rq = prq_pool.tile([P, 5, HD], BF16, tag="rq", name=f"rq_{tt}")
                for h in range(5):  # 4 q heads then k head
                    qk = 0 if h < QH else 4  # table offset (q vs k)
                    xn = p1.tile([P, HD], F32, tag="xn")
                    nc.vector.tensor_scalar(
                        xn[:], xq[:, h, :], rstd[:, ds(h, 1)],
                        nbias[:, ds(h, 1)], op0=ALU.mult, op1=ALU.add,
                    )
                    # rope with LN w folded into the tables:
                    #   re = xe*(we*cos) - xo*(wo*sin)
                    #   ro = xe*(we*sin) + xo*(wo*cos)
                    xr = xn.rearrange("p (f two) -> p two f", two=2)
                    xe = xr[:, 0, :]
                    xo = xr[:, 1, :]
                    ta = p1.tile([P, HALF], F32, tag="ta")
                    tb = p1.tile([P, HALF], F32, tag="tb")
                    rqr = rq[:, h, :].rearrange("p (f two) -> p two f", two=2)
                    nc.vector.tensor_mul(ta[:], xe, rope_sb[:, tt % NSLOT, qk + 0, :])
                    nc.vector.tensor_mul(tb[:], xo, rope_sb[:, tt % NSLOT, qk + 1, :])
                    if has_bias:
                        nc.vector.tensor_sub(ta[:], ta[:], tb[:])
                        nc.vector.tensor_add(
                            rqr[:, 0, :], ta[:], bias_sb[:, tt % NSLOT, (qk // 2), :]
                        )
                    else:
                        nc.vector.tensor_sub(rqr[:, 0, :], ta[:], tb[:])
                    nc.vector.tensor_mul(ta[:], xe, rope_sb[:, tt % NSLOT, qk + 2, :])
                    nc.vector.tensor_mul(tb[:], xo, rope_sb[:, tt % NSLOT, qk + 3, :])
                    if has_bias:
                        nc.vector.tensor_add(ta[:], ta[:], tb[:])
                        nc.vector.tensor_add(
                            rqr[:, 1, :], ta[:], bias_sb[:, tt % NSLOT, (qk // 2) + 1, :]
                        )
                    else:
                        nc.vector.tensor_add(rqr[:, 1, :], ta[:], tb[:])
                # [tok, head*hd] -> [hd, head, tok] on the DMA crossbar
                nc.sync.dma_start_transpose(qkT[:, :, ts(tt, P)], rq[:])

            rope_pend = []
            for pair in range(NT // 2):
                t0, t1 = 2 * pair, 2 * pair + 1
                xt_tiles = xt_cache.pop(pair) if pair in xt_cache else load_xt(pair)
                # prefetch next pair's x and rope tables
                if pair + 1 < NT // 2:
                    xt_cache[pair + 1] = load_xt(pair + 1, eng=nc.scalar)
                    load_tables(pair + 1, nc.scalar)

                pq0 = psum1.tile([P, FEAT], F32, tag="pqkv", name=f"pq_{t0}")
                pq1 = psum1.tile([P, FEAT], F32, tag="pqkv", name=f"pq_{t1}")
                # interleave both token tiles inside one k sweep so the
                # bootstrap (DMA-bound) period keeps the PE busy per chunk
                for k in range(KC):
                    st = k == 0
                    sp = k == KC - 1
                    for pq, sub in ((pq0, 0), (pq1, 1)):
                        lhsT = xt_tiles[:, k, ds(sub * P, P)]
                        nc.tensor.matmul(
                            pq[:, 0:512], lhsT, wqkvT_sb[:, k, 0:512],
                            start=st, stop=sp,
                        )
                        nc.tensor.matmul(
                            pq[:, 512:FEAT], lhsT, wqkvT_sb[:, k, 512:FEAT],
                            start=st, stop=sp,
                        )

                # stats first, rope deferred one pair: the DVE queue then
                # reaches the last tile's LN sqrt ~immediately after the last
                # QKV matmul instead of behind two tiles of rope work
                for tt, pq in ((t0, pq0), (t1, pq1)):
                    rope_pend.append((tt, *emit_stats(tt, pq)))
                while len(rope_pend) > 2:
                    emit_rope(*rope_pend.pop(0))
            while rope_pend:
                emit_rope(*rope_pend.pop(0))

        # ---------------- Phase 2: attention (+ per-head AllGather) ----------
        with (
            tc.tile_pool(name="w3", bufs=1) as w3,
            tc.tile_pool(name="p3", bufs=33) as p3,
            tc.tile_pool(name="p3o", bufs=3) as p3o,
            tc.tile_pool(name="paoT", bufs=4) as paoT,
        ):
            ao0 = [None] * KC  # first-half ao tiles, prefetched per head
            with (
                tc.tile_pool(name="p2", bufs=2) as p2,
                tc.tile_pool(name="paob", bufs=6) as paob,
                tc.tile_pool(name="psum_s", bufs=2, space="PSUM") as psum_s_pool,
                tc.tile_pool(name="psum_o", bufs=2, space="PSUM") as psum_o_pool,
                tc.tile_pool(name="psum_t2", bufs=2, space="PSUM") as psum_t2,
            ):
                def emit_scores(h, j, ve):
                    nkb = 4 * (j + 1)
                    if ve is nc.gpsimd and j == 0:
                        # early span: right-sized tile from the pre-reserved
                        # pool (no WAR against phase-1's deferred rope reads)
                        attn = p2e.tile([P, nkb, QSPAN], BF16, tag=f"attn{j}",
                                        name=f"attn_{h}_{j}")
                    else:
                        attn = p2.tile([P, NT, QSPAN], BF16, tag="attn",
                                       name=f"attn_{h}_{j}")
                    tri = masks_sb[:, 0, 0:P]
                    for ip in range(nkb // 2):
                        i = 2 * ip
                        ps = psum_s_pool.tile([P, 2, QSPAN], F32, tag="ps")
                        for u in range(2):
                            nc.tensor.matmul(
                                ps[:, u, :],
                                qkT[:, QH, ts(i + u, P)],
                                qkT[:, h, ds(j * QSPAN, QSPAN)],
                                start=True, stop=True,
                            )
                        # one exp over both blocks (amortize ACT fixed cost)
                        _ei = nc.scalar.activation(
                            attn[:, i : i + 2, :], ps[:], ACTF.Exp, scale=SCALE
                        )
                        del _ei  # unconstrained: the 2 extra table loads
                        # land in otherwise-idle ACT time at the transition
                        for u in range(2):
                            r = i + u - 4 * j
                            if 0 <= r < 4:
                                # mask only the true-diagonal 128x128 block:
                                # PV(q4) never reads tile i's columns q4 < r,
                                # so the off-diagonal garbage is never used
                                ve.tensor_mul(
                                    attn[:, i + u, ts(r, P)],
                                    attn[:, i + u, ts(r, P)],
                                    tri,
                                )
                    return attn

                def _post_head(h):
                    # bulk gather traffic rides the idle gpsimd DMA ring so it
                    # never delays the latency-critical sync-ring transposes;
                    # first token-halves land first so ao0 reads unblock early
                    if aps.get("no_collective"):
                        for r in range(NCORES):
                            nc.gpsimd.dma_start(
                                ag_out[h, ts(r, P), :], ag_in[ts(h, P), :]
                            )
                    else:
                        nc.gpsimd.collective_compute(
                            "AllGather",
                            ALU.bypass,
                            replica_groups=[list(range(NCORES))],
                            ins=[ag_in[ts(h, P), :]],
                            outs=[ag_out[h]],
                        )
                    # prefetch this head's first-half ao tiles for phase 3;
                    # phase-3 k order follows head-completion order (h3 first)
                    for r in range(NCORES):
                        k = (QH - 1 - h) * NCORES + r
                        a = p3.tile([P, T // 2], BF16, tag="ao", name=f"ao_0_{k}")
                        nc.sync.dma_start(
                            a[:], ag_out[h, ts(r, P), ds(0, T // 2)]
                        )
                        ao0[k] = a

                def emit_pv(h, j, attn, early):
                    aobt = paob.tile([P, 4, HD], BF16, tag="aobt",
                                     name=f"aobt_{h}_{j}")
                    pobs = []
                    for q4 in range(4):
                        qb = 4 * j + q4
                        po = psum_o_pool.tile([P, HD + 1], F32, tag="po")
                        for i in range(qb + 1):
                            nc.tensor.matmul(
                                po[:],
                                attn[:, i, ts(q4, P)],
                                vaug[:, i, :],
                                start=(i == 0), stop=(i == qb),
                            )
                        if early:
                            # evacuate via ACT: frees the po psum buffer
                            # without waiting on the DVE rope tail
                            pob = p2s.tile([P, HD + 1], F32, tag="pob",
                                           name=f"pob_{h}_{j}_{q4}")
                            nc.scalar.copy(pob[:], po[:])
                            pobs.append(pob)
                        else:
                            recip = p2s.tile([P, 1], F32, tag="recip")
                            nc.vector.reciprocal(recip[:], po[:, HD : HD + 1])
                            nc.vector.tensor_scalar_mul(
                                aobt[:, q4, :], po[:, 0:HD], recip[:]
                            )

                    def fin():
                        # normalize by the augmented-ones denominator column
                        for q4, pob in enumerate(pobs):
                            recip = p2s.tile([P, 1], F32, tag="recip")
                            nc.vector.reciprocal(recip[:], pob[:, HD : HD + 1])
                            nc.vector.tensor_scalar_mul(
                                aobt[:, q4, :], pob[:, 0:HD], recip[:]
                            )
                        if early:
                            # [qtok, blk*hd] -> [hd, blk, qtok] into aoTh
                            nc.sync.dma_start_transpose(
                                aoThs[h][:, ds(j * QSPAN, QSPAN)].rearrange(
                                    "p (b t) -> p b t", b=4
                                ),
                                aobt[:],
                            )
                        else:
                            # late spans transpose on the PE: cheap rows that
                            # double as p-state filler in the ACT-bound zone
                            for q4 in range(4):
                                pt2 = psum_t2.tile([P, P], BF16, tag="pt2")
                                nc.tensor.transpose(
                                    pt2[:], aobt[:, q4, :], ident_sb[:]
                                )
                                nc.vector.tensor_copy(
                                    aoThs[h][:, ts(4 * j + q4, P)], pt2[:]
                                )
                        if j == NQS - 1:
                            nc.sync.dma_start(ag_in[ts(h, P), :], aoThs[h][:])
                            _post_head(h)

                    return fin

                # j=0 spans for every head first (their deps are ready
                # earliest), then head-major so AllGathers fire early.
                from collections import deque

                spans = [(h, 0) for h in reversed(range(QH))] + [
                    (h, j) for h in reversed(range(QH)) for j in range(1, NQS)
                ]
                aoThs = {}
                for h in range(QH):
                    aoThs[h] = paoT.tile([P, T], BF16, tag="aoTh",
                                         name=f"aoT_{h}")
                # the first spans' masks ride the idle gpsimd engine and
                # their normalizes are deferred, so the phase-1 rope tail on
                # the DVE queue cannot block the start of attention
                N_EARLY = 6
                pv_q = deque()
                fin_q = deque()

                def step_pv():
                    h, j, attn, early = pv_q.popleft()
                    fin = emit_pv(h, j, attn, early)
                    if early:
                        fin_q.append(fin)
                    else:
                        while fin_q:  # preserve aoTh/ag ordering
                            fin_q.popleft()()
                        fin()

                woT_sb = None
                for idx, (h, j) in enumerate(spans):
                    early = idx < N_EARLY
                    attn = emit_scores(h, j, nc.gpsimd if early else nc.vector)
                    if idx == N_EARLY:
                        # prefetch wo weights while attention runs. Allocated
                        # after the first late-span attn tile so attn claims
                        # the freed wqkvT region (whose reads finished with
                        # the last QKV matmul) instead of a region still being
                        # read by the deferred rope tail.
                        woT_sb = w3.tile([P, KC, OUTC], BF16, tag="woT")
                        for k in range(KC):
                            nc.sync.dma_start(woT_sb[:, k, :], woT[ts(k, P), :])
                    pv_q.append((h, j, attn, early))
                    if len(pv_q) > 1:
                        step_pv()
                while pv_q:
                    step_pv()
                while fin_q:
                    fin_q.popleft()()

            # ---------------- Phase 3: output projection ----------------
            with tc.tile_pool(name="psum3", bufs=8, space="PSUM") as psum3:
                # prefetch all second-half ao tiles up front; they stream in
                # while the th=0 matmuls run. The last head's slices ride the
                # gpsimd ring (in-queue after its loopbacks) so the sync ring
                # never blocks waiting for them.
                ao1 = [None] * KC
                for k in range(KC):
                    g, r = divmod(k, NCORES)
                    h = QH - 1 - g
                    a = p3.tile([P, T // 2], BF16, tag="ao", name=f"ao_1_{k}")
                    eng = nc.gpsimd if h == 0 else nc.sync
                    eng.dma_start(
                        a[:], ag_out[h, ts(r, P), ds(T // 2, T // 2)]
                    )
                    ao1[k] = a

                def emit_out_group(th, cbg, cbs, ss):
                    """Accumulate psum tiles for (token-half th, col-blocks cbs,
                    tok-subspans ss) over all k, then evacuate."""
                    pos = {}
                    for cb in cbs:
                        for s2 in ss:
                            pos[(cb, s2)] = psum3.tile(
                                [P, 512], F32, tag="po3",
                                name=f"po3_{th}_{cb}_{s2}"
                            )
                    for k in range(KC):
                        a = ao0[k] if th == 0 else ao1[k]
                        for cb in cbs:
                            for s2 in ss:
                                nc.tensor.matmul(
                                    pos[(cb, s2)][:],
                                    woT_sb[:, k, ts(cb, P)],
                                    a[:, ts(s2, 512)],
                                    start=(k == 0), stop=(k == KC - 1),
                                )
                    for cb in cbs:
                        for s2 in ss:
                            ob = p3o.tile(
                                [P, 512], BF16, tag="ob",
                                name=f"ob_{th}_{cb}_{s2}"
                            )
                            # split evacuation across DVE and ACT so the
                            # final drain isn't serial on one engine
                            if s2 == ss[0]:
                                nc.vector.tensor_copy(ob[:], pos[(cb, s2)][:])
                            else:
                                nc.scalar.copy(ob[:], pos[(cb, s2)][:])
                            nc.sync.dma_start(
                                outT[ts(cb, P),
                                     ds(th * (T // 2) + s2 * 512, 512)],
                                ob[:],
                            )

                for th in range(2):  # token halves
                    for cbg in range(2):  # 2 col-block groups -> evac overlap
                        if th == 1 and cbg == 1:
                            # split the final group so earlier parts
                            # evacuate while later parts still compute
                            emit_out_group(th, cbg, [2], [0, 1])
                            emit_out_group(th, cbg, [3], [0])
                            emit_out_group(th, cbg, [3], [1])
                        else:
                            emit_out_group(th, cbg, [2 * cbg, 2 * cbg + 1],
                                           [0, 1])


def _build_program(no_collective=False, reps=1, has_bias=False):
    nc = bacc.Bacc(
        "TRN2",
        target_bir_lowering=False,
        debug=False,
        enable_asserts=True,
        num_devices=1 if no_collective else NCORES,
    )
    aps = {
        "xT": nc.dram_tensor("xT", [DIM, T], BF16, kind="ExternalInput").ap(),
        "wqkvT": nc.dram_tensor("wqkvT", [DIM, FEAT], BF16, kind="ExternalInput").ap(),
        "woT": nc.dram_tensor("woT", [NH * HD, OUTC], BF16, kind="ExternalInput").ap(),
        "ropeT": nc.dram_tensor(
            "ropeT", [P, NT, 8, HALF], F32, kind="ExternalInput"
        ).ap(),
        "masks": nc.dram_tensor("masks", [P, 4, QSPAN], BF16, kind="ExternalInput").ap(),
        "ident": nc.dram_tensor("ident", [P, P], BF16, kind="ExternalInput").ap(),
        "ag_in": nc.dram_tensor("ag_in", [QH * HD, T], BF16).ap(),
        "ag_out": nc.dram_tensor(
            "ag_out", [QH, NCORES * P, T], BF16, addr_space="Shared"
        ).ap(),
        "outT": nc.dram_tensor("outT", [OUTC, T], BF16, kind="ExternalOutput").ap(),
    }
    if has_bias:
        aps["biasT"] = nc.dram_tensor(
            "biasT", [P, NT, 4, HALF], F32, kind="ExternalInput"
        ).ap()
    aps["no_collective"] = no_collective
    aps["has_bias"] = has_bias
    with tile.TileContext(nc) as tc:
        aps["tc"] = tc
        for _rep in range(reps):
            _build_body(nc, aps)
    nc.compile()
    return nc


def get_program(has_bias=False):
    key = ("nc", has_bias)
    if key not in _PROGRAM_CACHE:
        _PROGRAM_CACHE[key] = _build_program(has_bias=has_bias)
    return _PROGRAM_CACHE[key]


def _rope_tables():
    """cos/sin tables computed exactly like the reference (jax fp32 on cpu)."""
    try:
        import jax

        cpu = jax.devices("cpu")[0]
        with jax.default_device(cpu):
            import jax.numpy as jnp

            inv_freq = 1.0 / (
                THETA ** (jnp.arange(HALF, dtype=jnp.float32) * 2.0 / HD)
            )
            pos = jnp.arange(T, dtype=jnp.float32)
            ang = pos[:, None] * inv_freq[None, :]
            cos = np.asarray(jnp.cos(ang), dtype=np.float32)
            sin = np.asarray(jnp.sin(ang), dtype=np.float32)
    except Exception:
        inv_freq = (
            1.0 / (THETA ** (np.arange(HALF, dtype=np.float32) * 2.0 / HD))
        ).astype(np.float32)
        ang = np.arange(T, dtype=np.float32)[:, None] * inv_freq[None, :]
        cos = np.cos(ang).astype(np.float32)
        sin = np.sin(ang).astype(np.float32)
    return cos, sin


def _make_const_inputs(q_ln_w, q_ln_b, k_ln_w, k_ln_b):
    cos, sin = _rope_tables()  # [T, HALF] f32
    cosP = cos.reshape(NT, P, HALF).transpose(1, 0, 2)  # [P, NT, HALF]
    sinP = sin.reshape(NT, P, HALF).transpose(1, 0, 2)

    qw = np.asarray(q_ln_w, np.float32)
    kw = np.asarray(k_ln_w, np.float32)
    qb = np.asarray(q_ln_b, np.float32)
    kb = np.asarray(k_ln_b, np.float32)

    # rope tables with LN weight folded in:
    #   [0] we*cos  [1] wo*sin  [2] we*sin  [3] wo*cos  (q),  [4..7] same (k)
    ropeT = np.zeros((P, NT, 8, HALF), np.float32)
    for base, w in ((0, qw), (4, kw)):
        ropeT[:, :, base + 0] = w[0::2] * cosP
        ropeT[:, :, base + 1] = w[1::2] * sinP
        ropeT[:, :, base + 2] = w[0::2] * sinP
        ropeT[:, :, base + 3] = w[1::2] * cosP

    has_bias = bool(np.any(qb != 0.0) or np.any(kb != 0.0))
    biasT = None
    if has_bias:
        # additive rope bias: [0] q re, [1] q ro, [2] k re, [3] k ro
        biasT = np.zeros((P, NT, 4, HALF), np.float32)
        for base, b in ((0, qb), (2, kb)):
            biasT[:, :, base + 0] = b[0::2] * cosP - b[1::2] * sinP
            biasT[:, :, base + 1] = b[0::2] * sinP + b[1::2] * cosP

    f = np.arange(QSPAN)[None, None, :]
    r = np.arange(4)[None, :, None]
    p = np.arange(P)[:, None, None]
    masks = (f >= 128 * r + p).astype(ml_dtypes.bfloat16)  # [P, 4, QSPAN]
    ident = np.eye(P, dtype=ml_dtypes.bfloat16)
    return ropeT, biasT, masks, ident, has_bias


# phase-3 lhsT rows are ordered (g, r, d) with k-group g covering local head
# h = QH-1-g (heads complete in reverse order), rank r; the ao feature order
# is (global head 4r+h, d). Permute woT rows to match.
_WOT_PERM = np.empty(NH * HD, np.int64)
for _g in range(QH):
    _h = QH - 1 - _g
    for _r in range(NCORES):
        _j = (_g * NCORES + _r) * HD
        _gl = (4 * _r + _h) * HD
        _WOT_PERM[_j : _j + HD] = np.arange(_gl, _gl + HD)


def make_in_maps(inputs):
    x = np.asarray(inputs["x"], dtype=ml_dtypes.bfloat16)
    wqkv = np.asarray(inputs["wqkv"], dtype=ml_dtypes.bfloat16)
    wo = np.asarray(inputs["wo"], dtype=ml_dtypes.bfloat16)
    q_ln_w = np.asarray(inputs["q_ln_w"], np.float32)
    q_ln_b = np.asarray(inputs["q_ln_b"], np.float32)
    k_ln_w = np.asarray(inputs["k_ln_w"], np.float32)
    k_ln_b = np.asarray(inputs["k_ln_b"], np.float32)

    ropeT, biasT, masks, ident, has_bias = _make_const_inputs(
        q_ln_w, q_ln_b, k_ln_w, k_ln_b
    )
    xT = np.ascontiguousarray(x.T)

    in_maps = []
    for c in range(NCORES):
        qrows = wqkv[c * QH * HD : (c + 1) * QH * HD]
        krows = wqkv[NH * HD + c * HD : NH * HD + (c + 1) * HD]
        vrows = wqkv[(NH + NKV) * HD + c * HD : (NH + NKV) * HD + (c + 1) * HD]
        wqkvT_c = np.ascontiguousarray(
            np.concatenate([qrows, krows, vrows], axis=0).T
        )
        woT_c = np.ascontiguousarray(
            wo[c * OUTC : (c + 1) * OUTC, :].T[_WOT_PERM, :]
        )
        m = {
            "xT": xT,
            "wqkvT": wqkvT_c,
            "woT": woT_c,
            "ropeT": ropeT,
            "masks": masks,
            "ident": ident,
        }
        if has_bias:
            m["biasT"] = biasT
        in_maps.append(m)
    return in_maps, has_bias


def kernel(**inputs):
    in_maps, has_bias = make_in_maps(inputs)
    nc = get_program(has_bias=has_bias)
    res = run_bass_kernel_spmd(nc, in_maps, list(range(NCORES)))
    outT_full = np.concatenate(
        [np.asarray(res.results[c]["outT"]) for c in range(NCORES)], axis=0
    )
    return np.ascontiguousarray(outT_full.T).astype(ml_dtypes.bfloat16)


if __name__ == "__main__":
    nc = get_program()
    print("program built ok")


# revision 56
# speedup vs baseline: 1.0130x; 1.0130x over previous
"""Trainium2 Bass kernel for fused sparse-attention block (nn_Attention_790273982568).

Full (unsharded) inputs in, full output out. Internally: tensor-parallel over
heads across 8 NeuronCores — each core owns 4 Q heads + 1 KV head (wqkv rows)
and 512 output columns of wo (rows of wo), with per-head on-device AllGathers
of the attention outputs (overlapped with attention) before the output
projection.

Key implementation notes:
- All [tok, hd] -> [hd, tok] transposes run on the DMA crossbar
  (dma_start_transpose), not the PE array.
- QKV projection interleaves the two token tiles of a pair inside one
  k-sweep so the DMA-bound bootstrap keeps the PE fed.
- Per-head LayerNorm stats are computed batched ([P, 5] per token tile:
  one Square, one Sqrt, one reciprocal) and the LN scale/bias plus rope
  cos/sin are folded into host-precomputed per-token tables.
- Attention spans are emitted j=0-first across heads (bridges the
  phase-1 -> phase-2 transition), then head-major so per-head AllGathers
  still overlap the remaining attention.
"""

import os
import sys

import numpy as np

for _p in ("/opt/trn_rl_repo", "/root/.axon_site/_ro/trn_rl_repo"):
    if _p not in sys.path and os.path.isdir(_p):
        sys.path.append(_p)

import ml_dtypes  # noqa: E402

import bass_rust as _bass_rust  # noqa: E402
import concourse.bass as bass  # noqa: E402
from concourse import bacc  # noqa: E402
import concourse.mybir as mybir  # noqa: E402
import concourse.tile as tile  # noqa: E402
from concourse.bass import ds, ts  # noqa: E402
from concourse.bass_utils import run_bass_kernel_spmd  # noqa: E402

# Problem shapes (hardcoded per spec)
T = 2048
DIM = 4096
HD = 128
NH = 32
NKV = 8
NCORES = 8
QH = NH // NCORES          # 4 q heads per core
FEAT = (QH + 2) * HD       # 768 qkv features per core
OUTC = DIM // NCORES       # 512 output columns per core
P = 128
NT = T // P                # 16 token tiles
KC = DIM // P              # 32 contraction chunks
QSPAN = 512
NQS = T // QSPAN           # 4 q spans
HALF = HD // 2
EPS = 1e-5
THETA = 10000.0
SCALE = 1.0 / float(np.sqrt(HD))

BF16 = mybir.dt.bfloat16
F32 = mybir.dt.float32
AX = mybir.AxisListType
ALU = mybir.AluOpType
ACTF = mybir.ActivationFunctionType

_PROGRAM_CACHE = {}


def _build_body(nc, aps):
    xT = aps["xT"]
    wqkvT = aps["wqkvT"]
    woT = aps["woT"]
    ropeT = aps["ropeT"]
    biasT = aps.get("biasT")
    masks = aps["masks"]
    ag_in = aps["ag_in"]
    ag_out = aps["ag_out"]       # [QH, NCORES*P, T]
    outT = aps["outT"]
    tc = aps["tc"]
    has_bias = aps["has_bias"]

    with (
        tc.tile_pool(name="consts", bufs=1) as consts,
        # pre-reserved pools for the early attention spans: their tiles must
        # NOT reuse SBUF whose last readers are phase-1's deferred rope ops,
        # or the exps pick up a write-after-read hazard on the rope tail
        tc.tile_pool(name="p2e", bufs=2) as p2e,
        tc.tile_pool(name="p2s", bufs=26) as p2s,
    ):
        masks_sb = consts.tile([P, 4, QSPAN], BF16, tag="masks")
        ident_sb = consts.tile([P, P], BF16, tag="ident")
        nc.gpsimd.dma_start(ident_sb[:], aps["ident"][:, :])

        # persistent activation strips
        qkT = consts.tile([P, QH + 1, T], BF16, tag="qkT")       # [hd, head, tok]
        vaug = consts.tile([P, NT, HD + 1], BF16, tag="vaug")    # [ktok%, ktile, hd+1]

        # ---------------- Phase 1: QKV projection + LN + RoPE ----------------
        with (
            tc.tile_pool(name="wq", bufs=1) as wq_pool,
            tc.tile_pool(name="p1", bufs=4) as p1,
            tc.tile_pool(name="p1sq", bufs=2) as p1sq,
            tc.tile_pool(name="px", bufs=2) as px,
            tc.tile_pool(name="p1s", bufs=6) as p1s,
            tc.tile_pool(name="prq", bufs=3) as prq_pool,
            tc.tile_pool(name="psum1", bufs=4, space="PSUM") as psum1,
        ):
            # 8-slot ring of per-tile rope tables (saves 16KB/partition)
            NSLOT = 8
            rope_sb = wq_pool.tile([P, NSLOT, 8, HALF], F32, tag="rope")
            if has_bias:
                bias_sb = wq_pool.tile([P, NSLOT, 4, HALF], F32, tag="biasT")

            def load_xt(pair, eng=None):
                tiles = px.tile([P, KC, 2 * P], BF16, tag="xt", name=f"xt_{pair}")
                for g in range(KC // 4):
                    (eng or nc.sync).dma_start(
                        tiles[:, ds(4 * g, 4), :],
                        xT[ds(4 * g * P, 4 * P), ds(pair * 2 * P, 2 * P)].rearrange(
                            "(k p) c -> p k c", p=P
                        ),
                    )
                return tiles

            def load_tables(pair, eng):
                slot = (2 * pair) % NSLOT
                eng.dma_start(
                    rope_sb[:, ds(slot, 2), :, :],
                    ropeT[:, ds(2 * pair, 2), :, :],
                )
                if has_bias:
                    eng.dma_start(
                        bias_sb[:, ds(slot, 2), :, :],
                        biasT[:, ds(2 * pair, 2), :, :],
                    )

            # stripe x/weight chunks across both HWDGE rings, k-interleaved,
            # so matmul k can start as soon as chunk k has landed
            xt0 = px.tile([P, KC, 2 * P], BF16, tag="xt", name="xt_0")
            wqkvT_sb = wq_pool.tile([P, KC, FEAT], BF16, tag="wqkvT")
            # single-chunk groups first so matmul k=0 starts ~3x sooner
            groups = [(k, 1) for k in range(4)] + [
                (g0, 4) for g0 in range(4, KC, 4)
            ]
            for gi, (g0, gn) in enumerate(groups):
                e0, e1 = (nc.sync, nc.scalar) if gi % 2 == 0 else (nc.scalar, nc.sync)
                e0.dma_start(
                    wqkvT_sb[:, ds(g0, gn), :],
                    wqkvT[ds(g0 * P, gn * P), :].rearrange("(k p) f -> p k f", p=P),
                )
                e1.dma_start(
                    xt0[:, ds(g0, gn), :],
                    xT[ds(g0 * P, gn * P), ds(0, 2 * P)].rearrange(
                        "(k p) c -> p k c", p=P
                    ),
                )
            load_tables(0, nc.gpsimd)
            nc.gpsimd.dma_start(masks_sb[:], masks[:, :, :])
            xt_cache = {0: xt0}

            def emit_stats(tt, pq):
                # v slice straight to vaug (round f32->bf16 exactly once).
                # The last pair's copies ride ACT so the transition-critical
                # LN stats aren't stuck behind the DVE queue.
                last = tt >= NT - 2
                cp = nc.scalar.copy if last else nc.vector.tensor_copy
                cp(vaug[:, tt, 0:HD], pq[:, 640:FEAT])
                nc.vector.memset(vaug[:, tt, HD : HD + 1], 1.0)
                # q/k slices as bf16 (match reference's bf16 xqkv)
                xq = p1.tile([P, 5, HD], BF16, tag="xq", name=f"xq_{tt}")
                cp(xq[:], pq[:, 0 : 5 * HD])

                # batched LN stats for all 5 heads: [P, 5]
                sq = p1sq.tile([P, 5, HD], F32, tag="sq")
                nc.scalar.activation(sq[:], xq[:], ACTF.Square)
                s1 = p1s.tile([P, 5], F32, tag="s1")
                nc.vector.reduce_sum(s1[:], xq[:], axis=AX.X)
                ssq = p1s.tile([P, 5], F32, tag="ssq")
                nc.vector.reduce_sum(ssq[:], sq[:], axis=AX.X)
                negmu = p1s.tile([P, 5], F32, tag="negmu")
                nc.vector.tensor_scalar_mul(negmu[:], s1[:], -1.0 / HD)
                mu2 = p1s.tile([P, 5], F32, tag="mu2")
                nc.vector.tensor_mul(mu2[:], negmu[:], negmu[:])
                v1 = p1s.tile([P, 5], F32, tag="v1")
                nc.vector.tensor_scalar(
                    v1[:], ssq[:], 1.0 / HD, EPS, op0=ALU.mult, op1=ALU.add
                )
                varb = p1s.tile([P, 5], F32, tag="varb")
                nc.vector.tensor_sub(varb[:], v1[:], mu2[:])
                std = p1s.tile([P, 5], F32, tag="std")
                _si = nc.scalar.activation(std[:], varb[:], ACTF.Sqrt)
                aps["last_sqrt"] = _si.ins
                rstd = p1s.tile([P, 5], F32, tag="rstd", name=f"rstd_{tt}")
                nc.vector.reciprocal(rstd[:], std[:])
                nbias = p1s.tile([P, 5], F32, tag="nbias", name=f"nb_{tt}")
                nc.vector.tensor_mul(nbias[:], negmu[:], rstd[:])
                return xq, rstd, nbias

            def emit_rope(tt, xq, rstd, nbias):
                rq = prq_pool.tile([P, 5, HD], BF16, tag="rq", name=f"rq_{tt}")
                for h in range(5):  # 4 q heads then k head
                    qk = 0 if h < QH else 4  # table offset (q vs k)
                    xn = p1.tile([P, HD], F32, tag="xn")
                    nc.vector.tensor_scalar(
                        xn[:], xq[:, h, :], rstd[:, ds(h, 1)],
                        nbias[:, ds(h, 1)], op0=ALU.mult, op1=ALU.add,
                    )
                    # rope with LN w folded into the tables:
                    #   re = xe*(we*cos) - xo*(wo*sin)
                    #   ro = xe*(we*sin) + xo*(wo*cos)
                    xr = xn.rearrange("p (f two) -> p two f", two=2)
                    xe = xr[:, 0, :]
                    xo = xr[:, 1, :]
                    ta = p1.tile([P, HALF], F32, tag="ta")
                    tb = p1.tile([P, HALF], F32, tag="tb")
                    rqr = rq[:, h, :].rearrange("p (f two) -> p two f", two=2)
                    nc.vector.tensor_mul(ta[:], xe, rope_sb[:, tt % NSLOT, qk + 0, :])
                    nc.vector.tensor_mul(tb[:], xo, rope_sb[:, tt % NSLOT, qk + 1, :])
                    if has_bias:
                        nc.vector.tensor_sub(ta[:], ta[:], tb[:])
                        nc.vector.tensor_add(
                            rqr[:, 0, :], ta[:], bias_sb[:, tt % NSLOT, (qk // 2), :]
                        )
                    else:
                        nc.vector.tensor_sub(rqr[:, 0, :], ta[:], tb[:])
                    nc.vector.tensor_mul(ta[:], xe, rope_sb[:, tt % NSLOT, qk + 2, :])
                    nc.vector.tensor_mul(tb[:], xo, rope_sb[:, tt % NSLOT, qk + 3, :])
                    if has_bias:
                        nc.vector.tensor_add(ta[:], ta[:], tb[:])
                        nc.vector.tensor_add(
                            rqr[:, 1, :], ta[:], bias_sb[:, tt % NSLOT, (qk // 2) + 1, :]
                        )
                    else:
                        nc.vector.tensor_add(rqr[:, 1, :], ta[:], tb[:])
                # [tok, head*hd] -> [hd, head, tok] on the DMA crossbar
                nc.sync.dma_start_transpose(qkT[:, :, ts(tt, P)], rq[:])

            rope_pend = []
            for pair in range(NT // 2):
                t0, t1 = 2 * pair, 2 * pair + 1
                xt_tiles = xt_cache.pop(pair) if pair in xt_cache else load_xt(pair)
                # prefetch next pair's x and rope tables
                if pair + 1 < NT // 2:
                    xt_cache[pair + 1] = load_xt(pair + 1, eng=nc.scalar)
                    load_tables(pair + 1, nc.scalar)

                pq0 = psum1.tile([P, FEAT], F32, tag="pqkv", name=f"pq_{t0}")
                pq1 = psum1.tile([P, FEAT], F32, tag="pqkv", name=f"pq_{t1}")
                # interleave both token tiles inside one k sweep so the
                # bootstrap (DMA-bound) period keeps the PE busy per chunk
                for k in range(KC):
                    st = k == 0
                    sp = k == KC - 1
                    for pq, sub in ((pq0, 0), (pq1, 1)):
                        lhsT = xt_tiles[:, k, ds(sub * P, P)]
                        nc.tensor.matmul(
                            pq[:, 0:512], lhsT, wqkvT_sb[:, k, 0:512],
                            start=st, stop=sp,
                        )
                        nc.tensor.matmul(
                            pq[:, 512:FEAT], lhsT, wqkvT_sb[:, k, 512:FEAT],
                            start=st, stop=sp,
                        )

                # stats first, rope deferred one pair: the DVE queue then
                # reaches the last tile's LN sqrt ~immediately after the last
                # QKV matmul instead of behind two tiles of rope work
                for tt, pq in ((t0, pq0), (t1, pq1)):
                    rope_pend.append((tt, *emit_stats(tt, pq)))
                while len(rope_pend) > 2:
                    emit_rope(*rope_pend.pop(0))
            while rope_pend:
                emit_rope(*rope_pend.pop(0))

        # ---------------- Phase 2: attention (+ per-head AllGather) ----------
        with (
            tc.tile_pool(name="w3", bufs=1) as w3,
            tc.tile_pool(name="p3", bufs=33) as p3,
            tc.tile_pool(name="p3o", bufs=3) as p3o,
            tc.tile_pool(name="paoT", bufs=4) as paoT,
        ):
            ao0 = [None] * KC  # first-half ao tiles, prefetched per head
            with (
                tc.tile_pool(name="p2", bufs=2) as p2,
                tc.tile_pool(name="paob", bufs=6) as paob,
                tc.tile_pool(name="psum_s", bufs=2, space="PSUM") as psum_s_pool,
                tc.tile_pool(name="psum_o", bufs=2, space="PSUM") as psum_o_pool,
                tc.tile_pool(name="psum_t2", bufs=2, space="PSUM") as psum_t2,
            ):
                def emit_scores(h, j, ve):
                    nkb = 4 * (j + 1)
                    if ve is nc.gpsimd and j == 0:
                        # early span: right-sized tile from the pre-reserved
                        # pool (no WAR against phase-1's deferred rope reads)
                        attn = p2e.tile([P, nkb, QSPAN], BF16, tag=f"attn{j}",
                                        name=f"attn_{h}_{j}")
                    else:
                        attn = p2.tile([P, NT, QSPAN], BF16, tag="attn",
                                       name=f"attn_{h}_{j}")
                    tri = masks_sb[:, 0, 0:P]
                    for ip in range(nkb // 2):
                        i = 2 * ip
                        ps = psum_s_pool.tile([P, 2, QSPAN], F32, tag="ps")
                        for u in range(2):
                            nc.tensor.matmul(
                                ps[:, u, :],
                                qkT[:, QH, ts(i + u, P)],
                                qkT[:, h, ds(j * QSPAN, QSPAN)],
                                start=True, stop=True,
                            )
                        # one exp over both blocks (amortize ACT fixed cost)
                        _ei = nc.scalar.activation(
                            attn[:, i : i + 2, :], ps[:], ACTF.Exp, scale=SCALE
                        )
                        del _ei  # unconstrained: the 2 extra table loads
                        # land in otherwise-idle ACT time at the transition
                        for u in range(2):
                            r = i + u - 4 * j
                            if 0 <= r < 4:
                                # mask only the true-diagonal 128x128 block:
                                # PV(q4) never reads tile i's columns q4 < r,
                                # so the off-diagonal garbage is never used
                                ve.tensor_mul(
                                    attn[:, i + u, ts(r, P)],
                                    attn[:, i + u, ts(r, P)],
                                    tri,
                                )
                    return attn

                def _post_head(h):
                    # bulk gather traffic rides the idle gpsimd DMA ring so it
                    # never delays the latency-critical sync-ring transposes;
                    # first token-halves land first so ao0 reads unblock early
                    if aps.get("no_collective"):
                        for r in range(NCORES):
                            nc.gpsimd.dma_start(
                                ag_out[h, ts(r, P), :], ag_in[ts(h, P), :]
                            )
                    else:
                        nc.gpsimd.collective_compute(
                            "AllGather",
                            ALU.bypass,
                            replica_groups=[list(range(NCORES))],
                            ins=[ag_in[ts(h, P), :]],
                            outs=[ag_out[h]],
                        )
                    # prefetch this head's first-half ao tiles for phase 3;
                    # phase-3 k order follows head-completion order (h3 first)
                    for r in range(NCORES):
                        k = (QH - 1 - h) * NCORES + r
                        a = p3.tile([P, T // 2], BF16, tag="ao", name=f"ao_0_{k}")
                        nc.sync.dma_start(
                            a[:], ag_out[h, ts(r, P), ds(0, T // 2)]
                        )
                        ao0[k] = a

                def emit_pv(h, j, attn, early):
                    aobt = paob.tile([P, 4, HD], BF16, tag="aobt",
                                     name=f"aobt_{h}_{j}")
                    pobs = []
                    for q4 in range(4):
                        qb = 4 * j + q4
                        po = psum_o_pool.tile([P, HD + 1], F32, tag="po")
                        for i in range(qb + 1):
                            nc.tensor.matmul(
                                po[:],
                                attn[:, i, ts(q4, P)],
                                vaug[:, i, :],
                                start=(i == 0), stop=(i == qb),
                            )
                        if early:
                            # evacuate via ACT: frees the po psum buffer
                            # without waiting on the DVE rope tail
                            pob = p2s.tile([P, HD + 1], F32, tag="pob",
                                           name=f"pob_{h}_{j}_{q4}")
                            nc.scalar.copy(pob[:], po[:])
                            pobs.append(pob)
                        else:
                            recip = p2s.tile([P, 1], F32, tag="recip")
                            nc.vector.reciprocal(recip[:], po[:, HD : HD + 1])
                            nc.vector.tensor_scalar_mul(
                                aobt[:, q4, :], po[:, 0:HD], recip[:]
                            )

                    def fin():
                        # normalize by the augmented-ones denominator column
                        for q4, pob in enumerate(pobs):
                            recip = p2s.tile([P, 1], F32, tag="recip")
                            nc.vector.reciprocal(recip[:], pob[:, HD : HD + 1])
                            nc.vector.tensor_scalar_mul(
                                aobt[:, q4, :], pob[:, 0:HD], recip[:]
                            )
                        if early:
                            # [qtok, blk*hd] -> [hd, blk, qtok] into aoTh
                            nc.sync.dma_start_transpose(
                                aoThs[h][:, ds(j * QSPAN, QSPAN)].rearrange(
                                    "p (b t) -> p b t", b=4
                                ),
                                aobt[:],
                            )
                        else:
                            # late spans transpose on the PE: cheap rows that
                            # double as p-state filler in the ACT-bound zone
                            for q4 in range(4):
                                pt2 = psum_t2.tile([P, P], BF16, tag="pt2")
                                nc.tensor.transpose(
                                    pt2[:], aobt[:, q4, :], ident_sb[:]
                                )
                                nc.vector.tensor_copy(
                                    aoThs[h][:, ts(4 * j + q4, P)], pt2[:]
                                )
                        if j == NQS - 1:
                            nc.sync.dma_start(ag_in[ts(h, P), :], aoThs[h][:])
                            _post_head(h)

                    return fin

                # j=0 spans for every head first (their deps are ready
                # earliest), then head-major so AllGathers fire early.
                from collections import deque

                spans = [(h, 0) for h in reversed(range(QH))] + [
                    (h, j) for h in reversed(range(QH)) for j in range(1, NQS)
                ]
                aoThs = {}
                for h in range(QH):
                    aoThs[h] = paoT.tile([P, T], BF16, tag="aoTh",
                                         name=f"aoT_{h}")
                # the first spans' masks ride the idle gpsimd engine and
                # their normalizes are deferred, so the phase-1 rope tail on
                # the DVE queue cannot block the start of attention
                N_EARLY = 4
                pv_q = deque()
                fin_q = deque()

                def step_pv():
                    h, j, attn, early = pv_q.popleft()
                    fin = emit_pv(h, j, attn, early)
                    if early:
                        fin_q.append(fin)
                    else:
                        while fin_q:  # preserve aoTh/ag ordering
                            fin_q.popleft()()
                        fin()

                woT_sb = None
                for idx, (h, j) in enumerate(spans):
                    early = idx < N_EARLY
                    attn = emit_scores(h, j, nc.gpsimd if early else nc.vector)
                    if idx == N_EARLY:
                        # prefetch wo weights while attention runs. Allocated
                        # after the first late-span attn tile so attn claims
                        # the freed wqkvT region (whose reads finished with
                        # the last QKV matmul) instead of a region still being
                        # read by the deferred rope tail.
                        woT_sb = w3.tile([P, KC, OUTC], BF16, tag="woT")
                        for k in range(KC):
                            nc.sync.dma_start(woT_sb[:, k, :], woT[ts(k, P), :])
                    pv_q.append((h, j, attn, early))
                    if len(pv_q) > 1:
                        step_pv()
                while pv_q:
                    step_pv()
                while fin_q:
                    fin_q.popleft()()

            # ---------------- Phase 3: output projection ----------------
            with tc.tile_pool(name="psum3", bufs=8, space="PSUM") as psum3:
                # prefetch all second-half ao tiles up front; they stream in
                # while the th=0 matmuls run. The last head's slices ride the
                # gpsimd ring (in-queue after its loopbacks) so the sync ring
                # never blocks waiting for them.
                ao1 = [None] * KC
                for k in range(KC):
                    g, r = divmod(k, NCORES)
                    h = QH - 1 - g
                    a = p3.tile([P, T // 2], BF16, tag="ao", name=f"ao_1_{k}")
                    eng = nc.gpsimd if h == 0 else nc.sync
                    eng.dma_start(
                        a[:], ag_out[h, ts(r, P), ds(T // 2, T // 2)]
                    )
                    ao1[k] = a

                def emit_out_group(th, cbg, cbs, ss):
                    """Accumulate psum tiles for (token-half th, col-blocks cbs,
                    tok-subspans ss) over all k, then evacuate."""
                    pos = {}
                    for cb in cbs:
                        for s2 in ss:
                            pos[(cb, s2)] = psum3.tile(
                                [P, 512], F32, tag="po3",
                                name=f"po3_{th}_{cb}_{s2}"
                            )
                    for k in range(KC):
                        a = ao0[k] if th == 0 else ao1[k]
                        for cb in cbs:
                            for s2 in ss:
                                nc.tensor.matmul(
                                    pos[(cb, s2)][:],
                                    woT_sb[:, k, ts(cb, P)],
                                    a[:, ts(s2, 512)],
                                    start=(k == 0), stop=(k == KC - 1),
                                )
                    for cb in cbs:
                        for s2 in ss:
                            ob = p3o.tile(
                                [P, 512], BF16, tag="ob",
                                name=f"ob_{th}_{cb}_{s2}"
                            )
                            # split evacuation across DVE and ACT so the
                            # final drain isn't serial on one engine
                            if s2 == ss[0]:
                                nc.vector.tensor_copy(ob[:], pos[(cb, s2)][:])
                            else:
                                nc.scalar.copy(ob[:], pos[(cb, s2)][:])
                            nc.sync.dma_start(
                                outT[ts(cb, P),
                                     ds(th * (T // 2) + s2 * 512, 512)],
                                ob[:],
                            )

                for th in range(2):  # token halves
                    for cbg in range(2):  # 2 col-block groups -> evac overlap
                        if th == 1 and cbg == 1:
                            # split the final group so earlier parts
                            # evacuate while later parts still compute
                            emit_out_group(th, cbg, [2], [0, 1])
                            emit_out_group(th, cbg, [3], [0])
                            emit_out_group(th, cbg, [3], [1])
                        else:
                            emit_out_group(th, cbg, [2 * cbg, 2 * cbg + 1],
                                           [0, 1])


def _build_program(no_collective=False, reps=1, has_bias=False):
    nc = bacc.Bacc(
        "TRN2",
        target_bir_lowering=False,
        debug=False,
        enable_asserts=True,
        num_devices=1 if no_collective else NCORES,
    )
    aps = {
        "xT": nc.dram_tensor("xT", [DIM, T], BF16, kind="ExternalInput").ap(),
        "wqkvT": nc.dram_tensor("wqkvT", [DIM, FEAT], BF16, kind="ExternalInput").ap(),
        "woT": nc.dram_tensor("woT", [NH * HD, OUTC], BF16, kind="ExternalInput").ap(),
        "ropeT": nc.dram_tensor(
            "ropeT", [P, NT, 8, HALF], F32, kind="ExternalInput"
        ).ap(),
        "masks": nc.dram_tensor("masks", [P, 4, QSPAN], BF16, kind="ExternalInput").ap(),
        "ident": nc.dram_tensor("ident", [P, P], BF16, kind="ExternalInput").ap(),
        "ag_in": nc.dram_tensor("ag_in", [QH * HD, T], BF16).ap(),
        "ag_out": nc.dram_tensor(
            "ag_out", [QH, NCORES * P, T], BF16, addr_space="Shared"
        ).ap(),
        "outT": nc.dram_tensor("outT", [OUTC, T], BF16, kind="ExternalOutput").ap(),
    }
    if has_bias:
        aps["biasT"] = nc.dram_tensor(
            "biasT", [P, NT, 4, HALF], F32, kind="ExternalInput"
        ).ap()
    aps["no_collective"] = no_collective
    aps["has_bias"] = has_bias
    with tile.TileContext(nc) as tc:
        aps["tc"] = tc
        for _rep in range(reps):
            _build_body(nc, aps)
    nc.compile()
    return nc


def get_program(has_bias=False):
    key = ("nc", has_bias)
    if key not in _PROGRAM_CACHE:
        _PROGRAM_CACHE[key] = _build_program(has_bias=has_bias)
    return _PROGRAM_CACHE[key]


def _rope_tables():
    """cos/sin tables computed exactly like the reference (jax fp32 on cpu)."""
    try:
        import jax

        cpu = jax.devices("cpu")[0]
        with jax.default_device(cpu):
            import jax.numpy as jnp

            inv_freq = 1.0 / (
                THETA ** (jnp.arange(HALF, dtype=jnp.float32) * 2.0 / HD)
            )
            pos = jnp.arange(T, dtype=jnp.float32)
            ang = pos[:, None] * inv_freq[None, :]
            cos = np.asarray(jnp.cos(ang), dtype=np.float32)
            sin = np.asarray(jnp.sin(ang), dtype=np.float32)
    except Exception:
        inv_freq = (
            1.0 / (THETA ** (np.arange(HALF, dtype=np.float32) * 2.0 / HD))
        ).astype(np.float32)
        ang = np.arange(T, dtype=np.float32)[:, None] * inv_freq[None, :]
        cos = np.cos(ang).astype(np.float32)
        sin = np.sin(ang).astype(np.float32)
    return cos, sin


def _make_const_inputs(q_ln_w, q_ln_b, k_ln_w, k_ln_b):
    cos, sin = _rope_tables()  # [T, HALF] f32
    cosP = cos.reshape(NT, P, HALF).transpose(1, 0, 2)  # [P, NT, HALF]
    sinP = sin.reshape(NT, P, HALF).transpose(1, 0, 2)

    qw = np.asarray(q_ln_w, np.float32)
    kw = np.asarray(k_ln_w, np.float32)
    qb = np.asarray(q_ln_b, np.float32)
    kb = np.asarray(k_ln_b, np.float32)

    # rope tables with LN weight folded in:
    #   [0] we*cos  [1] wo*sin  [2] we*sin  [3] wo*cos  (q),  [4..7] same (k)
    ropeT = np.zeros((P, NT, 8, HALF), np.float32)
    for base, w in ((0, qw), (4, kw)):
        ropeT[:, :, base + 0] = w[0::2] * cosP
        ropeT[:, :, base + 1] = w[1::2] * sinP
        ropeT[:, :, base + 2] = w[0::2] * sinP
        ropeT[:, :, base + 3] = w[1::2] * cosP

    has_bias = bool(np.any(qb != 0.0) or np.any(kb != 0.0))
    biasT = None
    if has_bias:
        # additive rope bias: [0] q re, [1] q ro, [2] k re, [3] k ro
        biasT = np.zeros((P, NT, 4, HALF), np.float32)
        for base, b in ((0, qb), (2, kb)):
            biasT[:, :, base + 0] = b[0::2] * cosP - b[1::2] * sinP
            biasT[:, :, base + 1] = b[0::2] * sinP + b[1::2] * cosP

    f = np.arange(QSPAN)[None, None, :]
    r = np.arange(4)[None, :, None]
    p = np.arange(P)[:, None, None]
    masks = (f >= 128 * r + p).astype(ml_dtypes.bfloat16)  # [P, 4, QSPAN]
    ident = np.eye(P, dtype=ml_dtypes.bfloat16)
    return ropeT, biasT, masks, ident, has_bias


# phase-3 lhsT rows are ordered (g, r, d) with k-group g covering local head
# h = QH-1-g (heads complete in reverse order), rank r; the ao feature order
# is (global head 4r+h, d). Permute woT rows to match.
_WOT_PERM = np.empty(NH * HD, np.int64)
for _g in range(QH):
    _h = QH - 1 - _g
    for _r in range(NCORES):
        _j = (_g * NCORES + _r) * HD
        _gl = (4 * _r + _h) * HD
        _WOT_PERM[_j : _j + HD] = np.arange(_gl, _gl + HD)


def make_in_maps(inputs):
    x = np.asarray(inputs["x"], dtype=ml_dtypes.bfloat16)
    wqkv = np.asarray(inputs["wqkv"], dtype=ml_dtypes.bfloat16)
    wo = np.asarray(inputs["wo"], dtype=ml_dtypes.bfloat16)
    q_ln_w = np.asarray(inputs["q_ln_w"], np.float32)
    q_ln_b = np.asarray(inputs["q_ln_b"], np.float32)
    k_ln_w = np.asarray(inputs["k_ln_w"], np.float32)
    k_ln_b = np.asarray(inputs["k_ln_b"], np.float32)

    ropeT, biasT, masks, ident, has_bias = _make_const_inputs(
        q_ln_w, q_ln_b, k_ln_w, k_ln_b
    )
    xT = np.ascontiguousarray(x.T)

    in_maps = []
    for c in range(NCORES):
        qrows = wqkv[c * QH * HD : (c + 1) * QH * HD]
        krows = wqkv[NH * HD + c * HD : NH * HD + (c + 1) * HD]
        vrows = wqkv[(NH + NKV) * HD + c * HD : (NH + NKV) * HD + (c + 1) * HD]
        wqkvT_c = np.ascontiguousarray(
            np.concatenate([qrows, krows, vrows], axis=0).T
        )
        woT_c = np.ascontiguousarray(
            wo[c * OUTC : (c + 1) * OUTC, :].T[_WOT_PERM, :]
        )
        m = {
            "xT": xT,
            "wqkvT": wqkvT_c,
            "woT": woT_c,
            "ropeT": ropeT,
            "masks": masks,
            "ident": ident,
        }
        if has_bias:
            m["biasT"] = biasT
        in_maps.append(m)
    return in_maps, has_bias


def kernel(**inputs):
    in_maps, has_bias = make_in_maps(inputs)
    nc = get_program(has_bias=has_bias)
    res = run_bass_kernel_spmd(nc, in_maps, list(range(NCORES)))
    outT_full = np.concatenate(
        [np.asarray(res.results[c]["outT"]) for c in range(NCORES)], axis=0
    )
    return np.ascontiguousarray(outT_full.T).astype(ml_dtypes.bfloat16)


if __name__ == "__main__":
    nc = get_program()
    print("program built ok")
